# revision 12
# baseline (speedup 1.0000x reference)
"""Trainium2 Bass kernel for nn_CutoffModule (CBAM-style channel gate + topk gather).

Reference computation (per sample):
    avg/max spatial pooling -> shared 2-layer MLP -> sum -> sigmoid -> attn [C, D]
    per scale d: top-128 channels (sorted desc) -> gather those channels of x.

Sharding: data-parallel over N across 8 cores (4 samples/core); MLP weights
replicated. Entirely self-contained: hardcodes N=32, C=512, H=W=64, D=4, r=16.

Single-read design: x is loaded into SBUF exactly once (32 MiB/core of HBM
reads). Pooling consumes the resident tiles; after topk the OUTPUT rows are
written by indirect *scatter* DMAs directly from the resident x tiles, so x
is never re-read from DRAM (the baseline gathered from DRAM: 96 MiB total;
this is 64 MiB).

The scatter needs channel->output-row offsets (the inverse of the topk
permutation). That inverse is built with a DMA round trip through a small
internal DRAM buffer: scatter rank-values (d*128+k) to tmp[channel], then
read tmp back channel-indexed. Channels not selected for a scale keep the
1e9 prefill, exceed bounds_check, and are silently skipped by the DGE.

Notes:
- sigmoid is strictly monotonic, so top_k(sigmoid(y)) == top_k(y); the kernel
  ranks pre-sigmoid logits and never materializes the sigmoid.
- w2 is permuted host-side to d-major (w2p[:, d*512+c] = w2aug[:, c*D+d]) so
  scale-d logits land contiguously in PSUM and copy to vals row 32*d.
- max pooling runs on GpSimd so the vector engine is free for the topk chains.
"""

import numpy as np

import concourse.bacc as bacc
import concourse.bass as bass
import concourse.tile as tile
from concourse.tile import add_dep_helper
from concourse import mybir
from concourse.bass_utils import run_bass_kernel_spmd

# Problem constants (hardcoded per harness contract)
N_FULL = 32
C = 512
HW = 64 * 64          # 4096
D = 4                 # depth scales
BLOCK = C // D        # 128
HID = C // 16         # 32  (MLP hidden)
N_CORES = 8
NS = N_FULL // N_CORES  # 4 samples per core
P = 128               # SBUF partitions
CT = C // P           # 4 channel tiles per sample
NEG_FILL = -1e30
BIG = 1.0e9           # tmp prefill; survives +n*512 and u32 convert as OOB

F32 = mybir.dt.float32
U32 = mybir.dt.uint32

TMP_ROWS = NS * D * C  # 8192 rank cells, one per (n, d, channel)


def _build_program():
    nc = bacc.Bacc("TRN2", target_bir_lowering=False, debug=False)

    x_d = nc.dram_tensor("x", [NS * C, HW], F32, kind="ExternalInput").ap()
    w1_d = nc.dram_tensor("w1", [C, HID], F32, kind="ExternalInput").ap()
    b1_d = nc.dram_tensor("b1", [HID, 1], F32, kind="ExternalInput").ap()
    # w2p = d-major permutation of [W2; 2*b2] (K=33 folds both bias adds)
    w2_d = nc.dram_tensor("w2p", [HID + 1, C * D], F32, kind="ExternalInput").ap()
    ident_d = nc.dram_tensor("ident", [P, P], F32, kind="ExternalInput").ap()
    # basecol[32d] = d*512 (tmp row base per scale); rkval[k, d] = d*128 + k
    basecol_d = nc.dram_tensor("basecol", [P, 1], F32, kind="ExternalInput").ap()
    rkval_d = nc.dram_tensor("rkval", [P, D], F32, kind="ExternalInput").ap()
    out_d = nc.dram_tensor("out", [NS * C, HW], F32, kind="ExternalOutput").ap()
    tmp_d = nc.dram_tensor("tmp", [TMP_ROWS, 1], F32, kind="Internal").ap()

    with tile.TileContext(nc) as tc:
        with (
            tc.tile_pool(name="xin", bufs=10) as xpool,
            tc.tile_pool(name="small", bufs=1) as sm,
            tc.tile_pool(name="pyp", bufs=1, space="PSUM") as pypool,
            tc.tile_pool(name="php", bufs=2, space="PSUM") as phpool,
            tc.tile_pool(name="ptp", bufs=1, space="PSUM") as ptpool,
        ):
            # ---- constants / weights into SBUF (scalar ring) ----
            w1_sb = sm.tile([P, CT, HID], F32)   # chunk ct = channels ct*128..+128
            nc.scalar.dma_start(
                out=w1_sb[:], in_=w1_d.rearrange("(c p) m -> p c m", p=P)
            )
            w2_sb = sm.tile([HID + 1, C * D], F32)
            nc.scalar.dma_start(out=w2_sb[:], in_=w2_d)
            b1_sb = sm.tile([HID, 1], F32)
            nc.scalar.dma_start(out=b1_sb[:], in_=b1_d)
            ident_sb = sm.tile([P, P], F32)
            nc.scalar.dma_start(out=ident_sb[:], in_=ident_d)
            basecol_sb = sm.tile([P, 1], F32)
            nc.scalar.dma_start(out=basecol_sb[:], in_=basecol_d)
            rkval_sb = sm.tile([P, D], F32)
            nc.scalar.dma_start(out=rkval_sb[:], in_=rkval_d)

            # prefill tmp with BIG so unselected channels scatter OOB
            big_sb = sm.tile([P, TMP_ROWS // P], F32)
            nc.gpsimd.memset(big_sb[:], BIG)
            pre_tmp = nc.scalar.dma_start(
                out=tmp_d.rearrange("(p a) o -> p (a o)", p=P), in_=big_sb[:]
            )

            # pooling accumulators per sample: [P, ct, {avg, max}]
            pools = [sm.tile([P, CT, 2], F32, name=f"pools{n}") for n in range(NS)]
            scratch = sm.tile([P, HW], F32)

            # hw_t: zeros + ones row built once; cols 32d rows 0:32 rewritten
            # per sample
            hw_t = sm.tile([HID + 1, P], F32)
            nc.gpsimd.memset(hw_t[:], 0.0)
            nc.vector.memset(hw_t[HID : HID + 1, :], 1.0)

            # per-parity static tiles (sample n uses set n%2)
            vals = [[sm.tile([P, C], F32, name=f"vals{q}_{i}") for i in range(2)]
                    for q in range(2)]
            for q in range(2):
                for i in range(2):
                    nc.gpsimd.memset(vals[q][i][:], 0.0)
            maxv = [sm.tile([P, 8], F32, name=f"maxv{q}") for q in range(2)]
            tki = [sm.tile([P, BLOCK], U32, name=f"tki{q}") for q in range(2)]
            idxf = [sm.tile([P, BLOCK], F32, name=f"idxf{q}") for q in range(2)]
            idxT = [sm.tile([P, P], U32, name=f"idxT{q}") for q in range(2)]
            rk_sb = [sm.tile([P, D * CT], F32, name=f"rk{q}") for q in range(2)]
            off_f = [sm.tile([P, D * CT], F32, name=f"offf{q}") for q in range(2)]
            off_u = [sm.tile([P, D * CT], U32, name=f"offu{q}") for q in range(2)]

            xt = [[None] * CT for _ in range(NS)]

            def load_issue(n):
                # sync engine only issues loads; gated purely by buffer frees
                for ct in range(CT):
                    row0 = (n * CT + ct) * P
                    t = xpool.tile([P, HW], F32, tag="xt")
                    xt[n][ct] = t
                    nc.sync.dma_start(out=t[:], in_=x_d[row0 : row0 + P, :])

            def pool_consume(n):
                for ct in range(CT):
                    t = xt[n][ct]
                    # avg pool on ScalarE: accum_out sums copy(x * 1/HW)
                    nc.scalar.activation(
                        out=scratch[:],
                        in_=t[:],
                        func=mybir.ActivationFunctionType.Copy,
                        scale=1.0 / HW,
                        accum_out=pools[n][:, ct, 0:1],
                    )
                    # max pool on DVE (tensor_tensor_reduce faults the engine
                    # on HW; gpsimd rejects TensorTensor at codegen)
                    nc.vector.reduce_max(
                        out=pools[n][:, ct, 1:2],
                        in_=t[:],
                        axis=mybir.AxisListType.X,
                    )

            def mlp(n):
                """Logits for sample n -> vals[n%2][0] row 32d = scale-d logits."""
                ph = phpool.tile([HID, 2], F32, space="PSUM", tag="ph")
                for ct in range(CT):
                    nc.tensor.matmul(
                        out=ph[:],
                        lhsT=w1_sb[:, ct, :],
                        rhs=pools[n][:, ct, :],
                        start=(ct == 0),
                        stop=(ct == CT - 1),
                    )
                # relu(ph + b1) fused on vector: (ph + b1) max 0
                hT = sm.tile([HID, 2], F32, name=f"hT{n}")
                nc.vector.tensor_scalar(
                    out=hT[:],
                    in0=ph[:],
                    scalar1=b1_sb[:, 0:1],
                    scalar2=0.0,
                    op0=mybir.AluOpType.add,
                    op1=mybir.AluOpType.max,
                )
                hsum = sm.tile([HID, 1], F32, name=f"hsum{n}")
                nc.vector.tensor_add(out=hsum[:], in0=hT[:, 0:1], in1=hT[:, 1:2])
                for d in range(D):
                    nc.vector.tensor_copy(
                        out=hw_t[0:HID, 32 * d : 32 * d + 1], in_=hsum[:]
                    )

                py = pypool.tile([P, C * D], F32, space="PSUM", tag="py")
                for s in range(D):
                    sl = slice(s * C, (s + 1) * C)
                    nc.tensor.matmul(
                        out=py[:, sl], lhsT=hw_t[:], rhs=w2_sb[:, sl],
                        start=True, stop=True,
                    )
                # w2 is d-major: py[32d, d*512 + c] = logit(c, d)
                va = vals[n % 2][0]
                for d in range(D):
                    nc.vector.tensor_copy(
                        out=va[32 * d : 32 * d + 1, :],
                        in_=py[32 * d : 32 * d + 1, d * C : (d + 1) * C],
                    )

            def topk(n):
                q = n % 2
                topk_idx = tki[q]
                mx = maxv[q]
                cur, nxt = vals[q]
                for k in range(BLOCK // 8):
                    nc.vector.max(out=mx[:], in_=cur[:])
                    nc.vector.max_index(
                        out=topk_idx[:, 8 * k : 8 * k + 8],
                        in_max=mx[:],
                        in_values=cur[:],
                    )
                    if k < BLOCK // 8 - 1:
                        nc.vector.match_replace(
                            out=nxt[:], in_to_replace=mx[:], in_values=cur[:],
                            imm_value=NEG_FILL,
                        )
                        cur, nxt = nxt, cur

            def invert_and_scatter(n):
                """Rank round trip through tmp, then scatter x tiles to out."""
                q = n % 2
                # idxf = f32(topk_idx) + d*512 (per partition) + n*2048
                nc.gpsimd.tensor_copy(out=idxf[q][:], in_=tki[q][:])
                nc.gpsimd.tensor_scalar(
                    out=idxf[q][:],
                    in0=idxf[q][:],
                    scalar1=basecol_sb[:, 0:1],
                    scalar2=float(n * D * C),
                    op0=mybir.AluOpType.add,
                    op1=mybir.AluOpType.add,
                )
                pt = ptpool.tile([P, P], F32, space="PSUM", tag="pt")
                nc.tensor.transpose(out=pt[:], in_=idxf[q][:], identity=ident_sb[:])
                # gpsimd can't read PSUM; DVE does the f32 -> u32 copy
                nc.vector.tensor_copy(out=idxT[q][:], in_=pt[:])

                # scatter rank value d*128+k into tmp[channel cell]
                rss = []
                for d in range(D):
                    col = 32 * d
                    rs = nc.gpsimd.indirect_dma_start(
                        out=tmp_d[:, :],
                        out_offset=bass.IndirectOffsetOnAxis(
                            ap=idxT[q][:, col : col + 1], axis=0
                        ),
                        in_=rkval_sb[:, d : d + 1],
                        in_offset=None,
                    )
                    add_dep_helper(rs.ins, pre_tmp.ins, reason="after tmp prefill")
                    rss.append(rs)

                # read back channel-indexed ranks: col 4d+ct. On the scalar
                # ring: putting this on sync would deadlock (later load
                # issues queue ahead of it but need frees from the scatters
                # that transitively wait on this readback).
                rb = nc.scalar.dma_start(
                    out=rk_sb[q][:],
                    in_=tmp_d[n * D * C : (n + 1) * D * C, :].rearrange(
                        "(j p) o -> p (j o)", p=P
                    ),
                )
                for rs in rss:
                    add_dep_helper(rb.ins, rs.ins, reason="readback after scatter")

                # out row = rank + n*512 (OOB-huge for unselected channels);
                # on gpsimd: it must wait for rb before the x-scatters anyway
                nc.gpsimd.tensor_scalar_add(
                    off_f[q][:], rk_sb[q][:], float(n * C)
                )
                nc.gpsimd.tensor_copy(out=off_u[q][:], in_=off_f[q][:])  # ->u32

                for d in range(D):
                    for ct in range(CT):
                        nc.gpsimd.indirect_dma_start(
                            out=out_d[:, :],
                            out_offset=bass.IndirectOffsetOnAxis(
                                ap=off_u[q][:, D * d + ct : D * d + ct + 1], axis=0
                            ),
                            in_=xt[n][ct][:],
                            in_offset=None,
                            bounds_check=NS * C - 1,
                            oob_is_err=False,
                        )

            # emission order sets scheduler priority AND avoids in-stream
            # deadlock: gpsimd's maxes for sample n+2 must come after sample
            # n's x-scatters (which free the buffers its loads wait on).
            load_issue(0)
            pool_consume(0)
            load_issue(1)
            pool_consume(1)
            for n in range(NS):
                mlp(n)
                topk(n)
                if n + 2 < NS:
                    load_issue(n + 2)
                invert_and_scatter(n)
                if n + 2 < NS:
                    pool_consume(n + 2)

    nc.compile()
    return nc


_NC_CACHE = None


def _get_nc():
    global _NC_CACHE
    if _NC_CACHE is None:
        _NC_CACHE = _build_program()
    return _NC_CACHE


def _make_in_maps(x, W1, b1, W2, b2):
    x = np.ascontiguousarray(np.asarray(x, dtype=np.float32)).reshape(N_FULL, C, HW)
    W1 = np.asarray(W1, dtype=np.float32)
    b1 = np.asarray(b1, dtype=np.float32).reshape(HID, 1)
    W2 = np.asarray(W2, dtype=np.float32)
    b2 = np.asarray(b2, dtype=np.float32).reshape(1, C * D)
    w2aug = np.vstack([W2, 2.0 * b2])  # [33, C*D], col c*D + d
    # d-major permutation: w2p[:, d*C + c] = w2aug[:, c*D + d]
    w2p = np.ascontiguousarray(
        w2aug.reshape(HID + 1, C, D).transpose(0, 2, 1).reshape(HID + 1, C * D)
    )
    ident = np.eye(P, dtype=np.float32)
    basecol = np.zeros((P, 1), np.float32)
    for d in range(D):
        basecol[32 * d, 0] = d * C
    rkval = np.zeros((P, D), np.float32)
    for d in range(D):
        rkval[:, d] = d * BLOCK + np.arange(P)
    in_maps = []
    for core in range(N_CORES):
        shard = x[core * NS : (core + 1) * NS].reshape(NS * C, HW)
        in_maps.append(
            {
                "x": np.ascontiguousarray(shard),
                "w1": W1,
                "b1": b1,
                "w2p": w2p,
                "ident": ident,
                "basecol": basecol,
                "rkval": rkval,
            }
        )
    return in_maps


def run(inputs, trace=False, **kwargs):
    """Run the SPMD kernel; returns (full_output, BassKernelResults)."""
    nc = _get_nc()
    in_maps = _make_in_maps(
        inputs["x"], inputs["W1"], inputs["b1"], inputs["W2"], inputs["b2"]
    )
    res = run_bass_kernel_spmd(
        nc, in_maps, core_ids=list(range(N_CORES)), trace=trace, **kwargs
    )
    parts = [res.results[i]["out"].reshape(NS, C, 64, 64) for i in range(N_CORES)]
    out = np.concatenate(parts, axis=0)
    return out, res


def kernel(**inputs) -> np.ndarray:
    out, _ = run(inputs)
    return out


# revision 13
# speedup vs baseline: 1.6383x; 1.6383x over previous
"""Trainium2 Bass kernel for nn_CutoffModule (CBAM-style channel gate + topk gather).

Reference computation (per sample):
    avg/max spatial pooling -> shared 2-layer MLP -> sum -> sigmoid -> attn [C, D]
    per scale d: top-128 channels (sorted desc) -> gather those channels of x.

Sharding: data-parallel over N across 8 cores (4 samples/core); MLP weights
replicated. Entirely self-contained: hardcodes N=32, C=512, H=W=64, D=4, r=16.

Schedule (per core, 4 samples as two pairs):
- all 16 x-tile loads issue up-front on the sync ring; with bufs=7 and pooling
  consumers keeping pace the read stream runs at full HBM rate (~420 GB/s).
- avg pool on ScalarE (activation accum), max pool on DVE (reduce_max).
- one merged topk chain per PAIR of samples (8 active partitions) so chain
  cost is halved vs per-sample chains; pair-0's gathers+stores overlap the
  vector work for pair 1.
- gathers: indirect DMA (gpsimd SWDGE) from DRAM x; stores on the sync ring.

Notes:
- sigmoid is strictly monotonic, so top_k(sigmoid(y)) == top_k(y); the kernel
  ranks pre-sigmoid logits and never materializes the sigmoid.
- w2 is permuted host-side to d-major (w2p[:, d*512+c] = w2aug[:, c*D+d]) so
  scale-d logits land contiguously in PSUM; w2aug row 32 = 2*b2 folds both
  bias adds into the K=33 matmul.
- relu(ph + b1) is a fused DVE tensor_scalar (add then max 0), keeping the
  scalar engine free for the avg-pool pass.
"""

import numpy as np

import concourse.bacc as bacc
import concourse.bass as bass
import concourse.tile as tile
from concourse import mybir
from concourse.bass_utils import run_bass_kernel_spmd

# Problem constants (hardcoded per harness contract)
N_FULL = 32
C = 512
HW = 64 * 64          # 4096
D = 4                 # depth scales
BLOCK = C // D        # 128
HID = C // 16         # 32  (MLP hidden)
N_CORES = 8
NS = N_FULL // N_CORES  # 4 samples per core
P = 128               # SBUF partitions
CT = C // P           # 4 channel tiles per sample
NEG_FILL = -1e30

F32 = mybir.dt.float32
U32 = mybir.dt.uint32


def _build_program():
    nc = bacc.Bacc("TRN2", target_bir_lowering=False, debug=False)

    x_d = nc.dram_tensor("x", [NS * C, HW], F32, kind="ExternalInput").ap()
    w1_d = nc.dram_tensor("w1", [C, HID], F32, kind="ExternalInput").ap()
    b1_d = nc.dram_tensor("b1", [HID, 1], F32, kind="ExternalInput").ap()
    w2_d = nc.dram_tensor("w2p", [HID + 1, C * D], F32, kind="ExternalInput").ap()
    ident_d = nc.dram_tensor("ident", [P, P], F32, kind="ExternalInput").ap()
    nofs_d = nc.dram_tensor("nofs", [P, 2], F32, kind="ExternalInput").ap()
    out_d = nc.dram_tensor("out", [NS * C, HW], F32, kind="ExternalOutput").ap()

    with tile.TileContext(nc) as tc:
        with (
            tc.tile_pool(name="xin", bufs=7) as xpool,
            tc.tile_pool(name="gbuf", bufs=3) as gpool,
            tc.tile_pool(name="small", bufs=1) as sm,
            tc.tile_pool(name="pyp", bufs=1, space="PSUM") as pypool,
            tc.tile_pool(name="php", bufs=2, space="PSUM") as phpool,
            tc.tile_pool(name="ptp", bufs=2, space="PSUM") as ptpool,
        ):
            # ---- constants / weights into SBUF (scalar ring) ----
            w1_sb = sm.tile([P, CT, HID], F32)   # chunk ct = channels ct*128..+128
            nc.scalar.dma_start(
                out=w1_sb[:], in_=w1_d.rearrange("(c p) m -> p c m", p=P)
            )
            w2_sb = sm.tile([HID + 1, C * D], F32)
            nc.scalar.dma_start(out=w2_sb[:], in_=w2_d)
            b1_sb = sm.tile([HID, 1], F32)
            nc.scalar.dma_start(out=b1_sb[:], in_=b1_d)
            ident_sb = sm.tile([P, P], F32)
            nc.scalar.dma_start(out=ident_sb[:], in_=ident_d)
            nofs_sb = sm.tile([P, 2], F32)
            nc.scalar.dma_start(out=nofs_sb[:], in_=nofs_d)

            # pair pooling accumulators: [P, ct, {avg0, avg1, max0, max1}]
            pools = [sm.tile([P, CT, 4], F32, name=f"pools{pp}") for pp in range(2)]
            scratch = sm.tile([P, HW], F32)

            # hw_t: zeros + ones row built once; cols 32d+i rewritten per pair
            hw_t = sm.tile([HID + 1, P], F32)
            nc.gpsimd.memset(hw_t[:], 0.0)
            nc.vector.memset(hw_t[HID : HID + 1, :], 1.0)

            # per-pair topk tiles (rows at partition 32*d + i; rest zeroed)
            vals = [[sm.tile([P, C], F32, name=f"vals{pp}_{i}") for i in range(2)]
                    for pp in range(2)]
            for pp in range(2):
                for i in range(2):
                    nc.gpsimd.memset(vals[pp][i][:], 0.0)
            maxv = [sm.tile([P, 8], F32, name=f"maxv{pp}") for pp in range(2)]
            tki = [sm.tile([P, BLOCK], U32, name=f"tki{pp}") for pp in range(2)]
            idxf = [sm.tile([P, BLOCK], F32, name=f"idxf{pp}") for pp in range(2)]
            idxT = [sm.tile([P, P], U32, name=f"idxT{pp}") for pp in range(2)]

            xt = [[None] * CT for _ in range(NS)]

            def load_issue(n):
                for ct in range(CT):
                    row0 = (n * CT + ct) * P
                    t = xpool.tile([P, HW], F32, tag="xt")
                    xt[n][ct] = t
                    nc.sync.dma_start(out=t[:], in_=x_d[row0 : row0 + P, :])

            def pool_consume(n):
                pp, i = divmod(n, 2)
                for ct in range(CT):
                    t = xt[n][ct]
                    nc.scalar.activation(
                        out=scratch[:],
                        in_=t[:],
                        func=mybir.ActivationFunctionType.Copy,
                        scale=1.0 / HW,
                        accum_out=pools[pp][:, ct, i : i + 1],
                    )
                    nc.vector.reduce_max(
                        out=pools[pp][:, ct, 2 + i : 3 + i],
                        in_=t[:],
                        axis=mybir.AxisListType.X,
                    )

            def mlp_pair(pp):
                """Logits for samples {2pp, 2pp+1} -> vals[pp][0]."""
                ph = phpool.tile([HID, 4], F32, space="PSUM", tag="ph")
                for ct in range(CT):
                    nc.tensor.matmul(
                        out=ph[:],
                        lhsT=w1_sb[:, ct, :],
                        rhs=pools[pp][:, ct, :],
                        start=(ct == 0),
                        stop=(ct == CT - 1),
                    )
                # relu(ph + b1) fused on DVE; scalar engine stays on avg duty
                hTa = sm.tile([HID, 2], F32, name=f"hTa{pp}")
                hTm = sm.tile([HID, 2], F32, name=f"hTm{pp}")
                for hT, sl in ((hTa, slice(0, 2)), (hTm, slice(2, 4))):
                    nc.vector.tensor_scalar(
                        out=hT[:],
                        in0=ph[:, sl],
                        scalar1=b1_sb[:, 0:1],
                        scalar2=0.0,
                        op0=mybir.AluOpType.add,
                        op1=mybir.AluOpType.max,
                    )
                hsum = sm.tile([HID, 2], F32, name=f"hsum{pp}")
                nc.vector.tensor_add(out=hsum[:], in0=hTa[:], in1=hTm[:])
                for d in range(D):
                    nc.vector.tensor_copy(
                        out=hw_t[0:HID, 32 * d : 32 * d + 2], in_=hsum[:]
                    )

                py = pypool.tile([P, C * D], F32, space="PSUM", tag="py")
                for s in range(D):
                    sl = slice(s * C, (s + 1) * C)
                    nc.tensor.matmul(
                        out=py[:, sl], lhsT=hw_t[:], rhs=w2_sb[:, sl],
                        start=True, stop=True,
                    )
                # w2 is d-major: py[32d+i, d*512 + c] = logit(sample i, c, d)
                va = vals[pp][0]
                for d in range(D):
                    nc.vector.tensor_copy(
                        out=va[32 * d : 32 * d + 2, :],
                        in_=py[32 * d : 32 * d + 2, d * C : (d + 1) * C],
                    )

            def topk_pair(pp):
                topk_idx = tki[pp]
                mx = maxv[pp]
                cur, nxt = vals[pp]
                for k in range(BLOCK // 8):
                    nc.vector.max(out=mx[:], in_=cur[:])
                    nc.vector.max_index(
                        out=topk_idx[:, 8 * k : 8 * k + 8],
                        in_max=mx[:],
                        in_values=cur[:],
                    )
                    if k < BLOCK // 8 - 1:
                        nc.vector.match_replace(
                            out=nxt[:], in_to_replace=mx[:], in_values=cur[:],
                            imm_value=NEG_FILL,
                        )
                        cur, nxt = nxt, cur
                # idxT column 32d+i = topk channels (+ sample base) on partitions
                nc.vector.tensor_copy(out=idxf[pp][:], in_=topk_idx[:])
                nc.vector.tensor_scalar_add(
                    idxf[pp][:], idxf[pp][:], nofs_sb[:, pp : pp + 1]
                )
                pt = ptpool.tile([P, P], F32, space="PSUM", tag="pt")
                nc.tensor.transpose(out=pt[:], in_=idxf[pp][:], identity=ident_sb[:])
                nc.vector.tensor_copy(out=idxT[pp][:], in_=pt[:])

            def gather_store_pair(pp):
                for i, n in enumerate((2 * pp, 2 * pp + 1)):
                    for d in range(D):
                        g = gpool.tile([P, HW], F32, tag="g")
                        nc.gpsimd.indirect_dma_start(
                            out=g[:],
                            out_offset=None,
                            in_=x_d[:, :],
                            in_offset=bass.IndirectOffsetOnAxis(
                                ap=idxT[pp][:, 32 * d + i : 32 * d + i + 1], axis=0
                            ),
                        )
                        o0 = n * C + d * BLOCK
                        nc.sync.dma_start(out=out_d[o0 : o0 + BLOCK, :], in_=g[:])

            # all loads issue first (sync ring is otherwise idle until stores);
            # pair-0's gathers+stores overlap pair-1's pooling + topk.
            for n in range(NS):
                load_issue(n)
            pool_consume(0)
            pool_consume(1)
            mlp_pair(0)
            topk_pair(0)
            gather_store_pair(0)
            pool_consume(2)
            pool_consume(3)
            mlp_pair(1)
            topk_pair(1)
            gather_store_pair(1)

    nc.compile()
    return nc


_NC_CACHE = None


def _get_nc():
    global _NC_CACHE
    if _NC_CACHE is None:
        _NC_CACHE = _build_program()
    return _NC_CACHE


def _make_in_maps(x, W1, b1, W2, b2):
    x = np.ascontiguousarray(np.asarray(x, dtype=np.float32)).reshape(N_FULL, C, HW)
    W1 = np.asarray(W1, dtype=np.float32)
    b1 = np.asarray(b1, dtype=np.float32).reshape(HID, 1)
    W2 = np.asarray(W2, dtype=np.float32)
    b2 = np.asarray(b2, dtype=np.float32).reshape(1, C * D)
    w2aug = np.vstack([W2, 2.0 * b2])  # [33, C*D], col c*D + d
    # d-major permutation: w2p[:, d*C + c] = w2aug[:, c*D + d]
    w2p = np.ascontiguousarray(
        w2aug.reshape(HID + 1, C, D).transpose(0, 2, 1).reshape(HID + 1, C * D)
    )
    ident = np.eye(P, dtype=np.float32)
    # partition 32d+i -> topk row (d, sample 2*pp+i): x row base = n*512
    pidx = np.arange(P)
    nofs = np.zeros((P, 2), np.float32)
    for pp in range(2):
        nofs[:, pp] = np.where(pidx % 32 < 2, (2 * pp + pidx % 32) * C, 0)
    in_maps = []
    for core in range(N_CORES):
        shard = x[core * NS : (core + 1) * NS].reshape(NS * C, HW)
        in_maps.append(
            {
                "x": np.ascontiguousarray(shard),
                "w1": W1,
                "b1": b1,
                "w2p": w2p,
                "ident": ident,
                "nofs": nofs,
            }
        )
    return in_maps


def run(inputs, trace=False, **kwargs):
    """Run the SPMD kernel; returns (full_output, BassKernelResults)."""
    nc = _get_nc()
    in_maps = _make_in_maps(
        inputs["x"], inputs["W1"], inputs["b1"], inputs["W2"], inputs["b2"]
    )
    res = run_bass_kernel_spmd(
        nc, in_maps, core_ids=list(range(N_CORES)), trace=trace, **kwargs
    )
    parts = [res.results[i]["out"].reshape(NS, C, 64, 64) for i in range(N_CORES)]
    out = np.concatenate(parts, axis=0)
    return out, res


def kernel(**inputs) -> np.ndarray:
    out, _ = run(inputs)
    return out
